# revision 4
# baseline (speedup 1.0000x reference)
"""Trainium2 Bass kernel for nn_CAM: channel attention (CAM) block. v2.

y = gamma * gelu(conv3x3(attn(x))) + x   with
  q/k/v = 1x1 conv projections (d = C/8 = 32),
  energy[d,e] = sum_n q[d,n] k[e,n],
  attn = softmax(max(energy) - energy, axis=e) == softmax(-energy),
  out  = attn @ v.

Sharding: 8 cores, 2 per sample (B=4); 64 own rows + 1 halo row each;
bottom-half cores get a vertically flipped tile + dy-flipped conv weight so
the SPMD program is identical; energy partials combined with a pairwise
AllReduce (4 KB).

v2 structure (vs the v1 baseline):
  * attention folded into the conv weights (w'[c,e,tap] = sum_d w[c,d,tap]
    attn[d,e]) so the 3x3 conv consumes V directly; pa3 (the padded conv
    input) is built during the load/QK phase, off the post-CC critical path,
    and the attn@V pass disappears.
  * x is loaded as raw f32 bits into F32R tiles (concurrent SWDGE DMAs; no
    f32r rounding - both consumers quantize to bf16 anyway, and the residual
    read gets exact x via bitcast).
  * QK is cast to a SINGLE bf16 (no hi/lo split): halves the DVE cast work,
    the transpose bytes, and the energy matmuls.  Measured end-to-end rel
    err 9e-4 vs the 2e-2 budget.
  * transposes ride the idle Sync HWDGE ring; stores ride the Scalar ring;
    pa3 writes go 2xACT + 1xDVE straight from the V PSUM (dx shifts are
    in-row column shifts + two strided edge memsets).
  * xr is double-buffered (xpool bufs=2) so iteration k+1's load overlaps
    iteration k's conv/store phase in the timing loop.
  * post-CC: softmax + 9 small matmuls build w' (ps_w bufs=2 pipelines the
    PSUM->SBUF casts) + conv (3 accumulating K=96 bf16 matmuls per [128,512]
    tile) + gelu (bf16 out) + gamma*out + x (DVE) + stores.

Known pitfalls hit (do not regress):
  * two matmul accumulation groups must NOT share a PSUM bank (corrupts
    accumulation data-dependently);
  * matmul PSUM outputs must start at partition 0;
  * stationary matmul operands allow only one free dim in their AP;
  * GPSIMD cannot read PSUM and lacks per-partition-scalar ops;
  * SBUF<->SBUF DMA serializes against DMA_TRANSPOSE (deadlock guard);
  * HWDGE rings drain FIFO per ring - serial loads underuse HBM, but
    concurrent SWDGE DMAs all complete together (~15us for 8.5MB).
"""
import sys

sys.path.insert(0, "/opt/trn_rl_repo")

from contextlib import ExitStack

import numpy as np
import ml_dtypes

import jax
from jax.sharding import Mesh, PartitionSpec, NamedSharding
from jax.experimental.shard_map import shard_map

import concourse.bacc as bacc
import concourse.tile as tile
from concourse import mybir
import concourse.bass as bass
from concourse.bass2jax import (
    _bass_exec_p,
    install_neuronx_cc_hook,
    partition_id_tensor,
)

F32 = mybir.dt.float32
F32R = mybir.dt.float32r
BF16 = mybir.dt.bfloat16
OP = mybir.AluOpType
AF = mybir.ActivationFunctionType

C = 256
D = 32
H = 128
W = 128
HE = 65          # rows per core incl. 1 halo row
NE = HE * W      # 8320
NOWN = 64 * W    # 8192 (rows owned by this core)
NB = 64          # 128-col blocks over own rows
N_CORES = 8
REPLICA_GROUPS = [[0, 1], [2, 3], [4, 5], [6, 7]]


def make_pools(tc, _ctx):
    return dict(
        consts=_ctx.enter_context(tc.tile_pool(name="consts", bufs=1)),
        big=_ctx.enter_context(tc.tile_pool(name="big", bufs=1)),
        xpool=_ctx.enter_context(tc.tile_pool(name="xpool", bufs=2)),
        work=_ctx.enter_context(tc.tile_pool(name="work", bufs=3)),
        small=_ctx.enter_context(tc.tile_pool(name="small", bufs=2)),
        ps_mm=_ctx.enter_context(tc.tile_pool(name="ps_mm", bufs=3, space="PSUM")),
        ps_qk=_ctx.enter_context(tc.tile_pool(name="ps_qk", bufs=2, space="PSUM")),
        ps_e=_ctx.enter_context(tc.tile_pool(name="ps_e", bufs=1, space="PSUM")),
        ps_w=_ctx.enter_context(tc.tile_pool(name="ps_w", bufs=2, space="PSUM")),
        dram=_ctx.enter_context(tc.tile_pool(name="dram", bufs=1, space="DRAM")),
    )


def _store_passthrough(nc, y_f, xr0, xr1):
    for t in range(16):
        for half, xh in ((0, xr0), (1, xr1)):
            nc.sync.dma_start(
                out=y_f[half * 128:(half + 1) * 128, 512 * t:512 * t + 512],
                in_=xh[:, 512 * t:512 * t + 512].bitcast(F32))


def build_body(tc, aps, pools, use_cc=True, parts=None):
    parts = parts or {"v", "qkt", "attn", "conv"}
    nc = tc.nc
    xe, wqkT, wvT, bqk, bvv, wpp, gam, y = (
        aps["xe"], aps["wqkT"], aps["wvT"], aps["bqk"], aps["bv"],
        aps["wpp"], aps["gamma"], aps["y"],
    )
    xe_f = xe.rearrange("c h w -> c (h w)")          # [256, 8320]
    y_f = y.rearrange("c h w -> c (h w)")            # [256, 8192]

    consts, big, work, small = (pools["consts"], pools["big"], pools["work"],
                                pools["small"])
    xpool = pools["xpool"]
    ps_mm, ps_qk, ps_e, dram = (pools["ps_mm"], pools["ps_qk"], pools["ps_e"],
                                pools["dram"])
    ps_w = pools["ps_w"]

    # ---- weights / constants (SWDGE DMA casts f32 -> f32r in flight) ----
    wqk_r = consts.tile([128, 2, 64], F32R, tag="wqkr")
    wv_r = consts.tile([128, 2, 32], F32R, tag="wvr")
    for c in range(2):
        nc.gpsimd.dma_start(out=wqk_r[:, c, :], in_=wqkT[c])
        nc.gpsimd.dma_start(out=wv_r[:, c, :], in_=wvT[c])
    bqk_sb = consts.tile([64, 1], F32)
    nc.sync.dma_start(
        out=bqk_sb[:],
        in_=bass.AP(tensor=bqk.tensor, offset=bqk.offset, ap=[[1, 64], [1, 1]]))
    bv3_sb = consts.tile([96, 1], F32)
    for k in range(3):
        nc.sync.dma_start(
            out=bv3_sb[32 * k:32 * (k + 1)],
            in_=bass.AP(tensor=bvv.tensor, offset=bvv.offset,
                        ap=[[1, 32], [1, 1]]))
    gam_sb = consts.tile([128, 1], F32)
    nc.sync.dma_start(
        out=gam_sb[:],
        in_=bass.AP(tensor=gam.tensor, offset=gam.offset, ap=[[0, 128], [1, 1]]))
    # w-prime source: [32 d, 3 dx, 3 dy, 256 c] bf16
    wpp_sb = consts.tile([32, 3, 3, 256], BF16)
    nc.sync.dma_start(out=wpp_sb[:], in_=wpp[:])

    # ---- x load: concurrent SWDGE DMAs (F32R bytes, no rounding) ----
    xr0 = xpool.tile([128, NE], F32R, tag="xr0")
    xr1 = xpool.tile([128, NE], F32R, tag="xr1")
    NCHUNK = 4
    csz = NE // NCHUNK  # 2080
    for j in range(NCHUNK):
        s = j * csz
        for xrh, lo in ((xr0, 0), (xr1, 128)):
            nc.gpsimd.dma_start(out=xrh[:, s:s + csz],
                                in_=xe_f[lo:lo + 128, s:s + csz])

    qk2 = big.tile([64, NOWN], BF16)                        # [ Q|K , n ] bf16
    qkt = big.tile([128, 64, 64], BF16)                     # transposed chunks
    pa3 = big.tile([96, 66, 128], BF16)                     # pre-shifted conv input
    nv = (NE + 511) // 512  # 17
    nc.vector.memset(pa3[:, 0, :], 0.0)          # top zero row (h=-1)

    # ---- fused QK+V per 512-col chunk; transpose + energy interleaved ----
    do_qk = "qkt" in parts
    e1t = ps_e.tile([32, 32], F32, tag="e1")
    for i in range(nv):
        s = i * 512
        w = min(512, NE - s)
        sl = slice(s, s + w)
        if do_qk and i < 16:
            qp = ps_qk.tile([64, 512], F32, tag="qk")
            nc.tensor.matmul(qp[:], wqk_r[:, 0, :], xr0[:, sl],
                             start=True, stop=False)
            nc.tensor.matmul(qp[:], wqk_r[:, 1, :], xr1[:, sl],
                             start=False, stop=True)
            # bias-add + bf16 cast (DVE)
            nc.vector.tensor_scalar(out=qk2[:, sl], in0=qp[:],
                                    scalar1=bqk_sb[:], scalar2=None, op0=OP.add)
        if "v" in parts:
            vp = ps_mm.tile([32, 512], F32, tag="mm")
            nc.tensor.matmul(vp[:, :w], wv_r[:, 0, :], xr0[:, sl],
                             start=True, stop=False)
            nc.tensor.matmul(vp[:, :w], wv_r[:, 1, :], xr1[:, sl],
                             start=False, stop=True)
            nh = w // 128
            r0 = s // 128
            vsrc = vp[:, :w].rearrange("p (h w) -> p h w", w=128)
            rows = slice(1 + r0, 1 + r0 + nh)
            # dx=1 (middle) + dx=2 (right) on ACT; dx=0 (left) on DVE
            nc.scalar.activation(out=pa3[32:64, rows, :], in_=vsrc,
                                 func=AF.Identity, bias=bv3_sb[0:32], scale=1.0)
            nc.scalar.activation(out=pa3[64:96, rows, 0:127],
                                 in_=vsrc[:, :, 1:128],
                                 func=AF.Identity, bias=bv3_sb[0:32], scale=1.0)
            nc.vector.tensor_scalar(out=pa3[0:32, rows, 1:128],
                                    in0=vsrc[:, :, 0:127],
                                    scalar1=bv3_sb[0:32], scalar2=None,
                                    op0=OP.add)
        if do_qk and i % 4 == 3:
            j = i // 4
            tsl = slice(j * 2048, (j + 1) * 2048)
            nc.sync.dma_start_transpose(
                qkt[:, j * 16:(j + 1) * 16, :], qk2[:, tsl])
    if do_qk:
        for b in range(NB):
            nc.tensor.matmul(e1t[:], qkt[:, b, 0:32], qkt[:, b, 32:64],
                             start=(b == 0), stop=(b == NB - 1))
    if "v" in parts:
        # zero the out-of-image columns (w=0 of dx=0 block, w=127 of dx=2)
        nc.vector.memset(pa3[0:32, 1:66, 0], 0.0)
        nc.vector.memset(pa3[64:96, 1:66, 127], 0.0)

    # ---- energy result to SBUF ----
    e_sb = small.tile([32, 32], F32, tag="esb")
    nc.vector.tensor_copy(out=e_sb[:], in_=e1t[:])

    # ---- AllReduce energy across the sample pair ----
    E_sb = small.tile([32, 32], F32, tag="Esb")
    if use_cc:
        ein = dram.tile([32, 32], F32)
        eout = dram.tile([32, 32], F32)
        nc.gpsimd.dma_start(out=ein[:], in_=e_sb[:])
        nc.gpsimd.collective_compute(
            "AllReduce", OP.add, replica_groups=REPLICA_GROUPS,
            ins=[ein.opt()], outs=[eout.opt()])
        nc.gpsimd.dma_start(out=E_sb[:], in_=eout[:])
    else:
        nc.gpsimd.tensor_copy(out=E_sb[:], in_=e_sb[:])

    # ---- softmax over e of -E, stable via min ----
    rmin = small.tile([32, 1], F32, tag="rmin")
    nc.vector.tensor_reduce(out=rmin[:], in_=E_sb[:], axis=mybir.AxisListType.X,
                            op=OP.min)
    t_sb = small.tile([32, 32], F32, tag="tsb")
    nc.vector.tensor_scalar(out=t_sb[:], in0=E_sb[:], scalar1=rmin[:],
                            scalar2=None, op0=OP.subtract)
    # exp(-t) ~= (1 + t/64)^-64 (6 DVE squarings + reciprocal; avoids the
    # ACT Exp table load; overflow -> inf -> 1/inf = 0 handles the tail)
    xp = small.tile([32, 32], F32, tag="xp")
    nc.vector.tensor_scalar(out=xp[:], in0=t_sb[:], scalar1=1.0 / 64.0,
                            scalar2=1.0, op0=OP.mult, op1=OP.add)
    for _ in range(6):
        nc.vector.tensor_tensor(out=xp[:], in0=xp[:], in1=xp[:], op=OP.mult)
    p_sb = small.tile([32, 32], F32, tag="psb")
    nc.vector.reciprocal(out=p_sb[:], in_=xp[:])
    ssum = small.tile([32, 1], F32, tag="ssum")
    nc.vector.reduce_sum(out=ssum[:], in_=p_sb[:], axis=mybir.AxisListType.X)
    rs = small.tile([32, 1], F32, tag="rs")
    nc.vector.reciprocal(out=rs[:], in_=ssum[:])
    attn_sb = small.tile([32, 32], F32, tag="attn")
    nc.vector.tensor_scalar(out=attn_sb[:], in0=p_sb[:], scalar1=rs[:],
                            scalar2=None, op0=OP.mult)
    attn_bf = small.tile([32, 32], BF16, tag="attnbf")
    nc.vector.tensor_copy(out=attn_bf[:], in_=attn_sb[:])

    if "dbg" in parts:
        nc.sync.dma_start(out=y_f[0:128, 0:4096].bitcast(BF16),
                          in_=qkt[:, :, :])
        nc.sync.dma_start(out=y_f[128:192, 0:8192].bitcast(BF16),
                          in_=qk2[:, :, :])
        nc.sync.dma_start(out=y_f[224:256, 0:32], in_=e_sb[:])
        return

    # ---- fold attention into conv weights: w2[(dx,e), dy, c] ----
    w2_sb = small.tile([96, 3, 256], BF16, tag="w2")
    for dy in range(3):
        for dx in range(3):
            wps = ps_w.tile([32, 256], F32, tag="wp")
            nc.tensor.matmul(wps[:], attn_bf[:], wpp_sb[:, dx, dy, :],
                             start=True, stop=True)
            if dx == 1:
                nc.scalar.activation(out=w2_sb[dx * 32:(dx + 1) * 32, dy, :],
                                     in_=wps[:], func=AF.Copy)
            else:
                nc.vector.tensor_copy(out=w2_sb[dx * 32:(dx + 1) * 32, dy, :],
                                      in_=wps[:])

    if "conv" not in parts:
        return _store_passthrough(nc, y_f, xr0, xr1)

    # ---- conv 3x3 (bf16) + exact gelu + gamma*out + x, then store ----
    for tg in range(4):
        for half in range(2):
            xh = xr0 if half == 0 else xr1
            yo4 = work.tile([128, 2048], F32, tag="yo")
            for tq in range(4):
                t = 4 * tg + tq
                cp = ps_mm.tile([128, 512], F32, tag="mm")
                for dy in range(3):
                    nc.tensor.matmul(
                        cp[:], w2_sb[:, dy, half * 128:(half + 1) * 128],
                        pa3[:, 4 * t + dy:4 * t + dy + 4, :],
                        start=(dy == 0), stop=(dy == 2))
                yt = work.tile([128, 512], BF16, tag="yt")
                nc.scalar.activation(out=yt[:], in_=cp[:], func=AF.Gelu)
                nc.vector.scalar_tensor_tensor(
                    out=yo4[:, tq * 512:(tq + 1) * 512], in0=yt[:],
                    scalar=gam_sb[:],
                    in1=xh[:, 512 * t:512 * t + 512].bitcast(F32),
                    op0=OP.mult, op1=OP.add)
            nc.scalar.dma_start(
                out=y_f[half * 128:(half + 1) * 128,
                        2048 * tg:2048 * (tg + 1)], in_=yo4[:])


def build_nc(loop_k=None, use_cc=True, trace_sim=False, parts=None,
             static_k=1):
    nc = bacc.Bacc("TRN2", target_bir_lowering=False, debug=False,
                   num_devices=N_CORES)
    aps = {
        "xe": nc.dram_tensor("xe", [C, HE, W], F32R, kind="ExternalInput").ap(),
        "wqkT": nc.dram_tensor("wqkT", [2, 128, 64], F32, kind="ExternalInput").ap(),
        "wvT": nc.dram_tensor("wvT", [2, 128, 32], F32, kind="ExternalInput").ap(),
        "bqk": nc.dram_tensor("bqk", [64], F32, kind="ExternalInput").ap(),
        "bv": nc.dram_tensor("bv", [D], F32, kind="ExternalInput").ap(),
        "wpp": nc.dram_tensor("wpp", [32, 3, 3, 256], BF16, kind="ExternalInput").ap(),
        "gamma": nc.dram_tensor("gamma", [1], F32, kind="ExternalInput").ap(),
        "y": nc.dram_tensor("y", [C, 64, W], F32, kind="ExternalOutput").ap(),
    }
    with tile.TileContext(nc, trace_sim=trace_sim) as tc:
        with ExitStack() as _ctx:
            pools = make_pools(tc, _ctx)
            if loop_k is None:
                for _ in range(static_k):
                    build_body(tc, aps, pools, use_cc, parts)
            else:
                with tc.For_i(0, loop_k, 1):
                    build_body(tc, aps, pools, use_cc, parts)
    nc.finalize()
    return nc


class SpmdRunner:
    def __init__(self, nc, n_cores):
        install_neuronx_cc_hook()
        self.nc = nc
        self.n_cores = n_cores
        partition_name = nc.partition_id_tensor.name if nc.partition_id_tensor else None
        in_names, out_names, out_avals, zero_outs = [], [], [], []
        for alloc in nc.m.functions[0].allocations:
            if not isinstance(alloc, mybir.MemoryLocationSet):
                continue
            name = alloc.memorylocations[0].name
            if alloc.kind == "ExternalInput":
                if name != partition_name:
                    in_names.append(name)
            elif alloc.kind == "ExternalOutput":
                shape = tuple(alloc.tensor_shape)
                dtype = mybir.dt.np(alloc.dtype)
                out_names.append(name)
                out_avals.append(jax.core.ShapedArray(shape, dtype))
                zero_outs.append(np.zeros(shape, dtype))
        self.in_names, self.out_names = in_names, out_names
        self.out_avals, self.zero_outs = out_avals, zero_outs
        self.n_params = len(in_names)
        all_in = list(in_names) + list(out_names)
        if partition_name is not None:
            all_in.append(partition_name)

        def _body(*args):
            operands = list(args)
            if partition_name is not None:
                operands.append(partition_id_tensor())
            return tuple(_bass_exec_p.bind(
                *operands, out_avals=tuple(out_avals), in_names=tuple(all_in),
                out_names=tuple(out_names), lowering_input_output_aliases=(),
                sim_require_finite=False, sim_require_nnan=False, nc=nc))

        devices = jax.devices()[:n_cores]
        self.mesh = Mesh(np.asarray(devices), ("core",))
        n_outs = len(out_avals)
        in_specs = (PartitionSpec("core"),) * (self.n_params + n_outs)
        out_specs = (PartitionSpec("core"),) * n_outs
        self.sharded = jax.jit(
            shard_map(_body, mesh=self.mesh, in_specs=in_specs,
                      out_specs=out_specs, check_rep=False),
            keep_unused=True)

    def prepare(self, in_maps):
        n = self.n_cores
        concat_in = [
            np.concatenate([np.asarray(in_maps[c][k]) for c in range(n)], axis=0)
            for k in self.in_names
        ]
        concat_zero = [np.zeros((n * z.shape[0], *z.shape[1:]), z.dtype)
                       for z in self.zero_outs]
        sh = NamedSharding(self.mesh, PartitionSpec("core"))
        return [jax.device_put(a, sh) for a in concat_in + concat_zero]

    def run(self, args):
        outs = self.sharded(*args)
        jax.block_until_ready(outs)
        return outs

    def results(self, outs):
        n = self.n_cores
        return [
            {name: np.asarray(outs[i]).reshape(n, *self.out_avals[i].shape)[c]
             for i, name in enumerate(self.out_names)}
            for c in range(n)
        ]


_RUNNER_CACHE = {}


def get_runner(loop_k=None, use_cc=True, parts=None, static_k=1):
    key = (loop_k, use_cc, tuple(sorted(parts)) if parts else None, static_k)
    if key not in _RUNNER_CACHE:
        _RUNNER_CACHE[key] = SpmdRunner(
            build_nc(loop_k, use_cc, parts=parts, static_k=static_k), N_CORES)
    return _RUNNER_CACHE[key]


def make_in_maps(x, wq, bq, wk, bk, wv, bv, wp, gamma):
    """Shard FULL inputs into 8 per-core input dicts (with flip trick)."""
    B = x.shape[0]
    wqkT = np.ascontiguousarray(
        np.concatenate([wq.T, wk.T], axis=1).reshape(2, 128, 64), np.float32)
    wvT = np.ascontiguousarray(wv.T.reshape(2, 128, 32), np.float32)
    bqk = np.concatenate([bq, bk]).astype(np.float32)
    # w-prime source layout: [d, dx, dy, c] = wp[c, d, dy, dx] (dy-flipped for
    # the bottom-half cores)
    wpp_n = np.ascontiguousarray(
        np.transpose(wp, (1, 3, 2, 0))).astype(ml_dtypes.bfloat16)
    wp_fl = wp[:, :, ::-1, :]
    wpp_f = np.ascontiguousarray(
        np.transpose(wp_fl, (1, 3, 2, 0))).astype(ml_dtypes.bfloat16)
    gam = gamma.astype(np.float32)
    bvf = bv.astype(np.float32)

    in_maps = []
    for b in range(B):
        top = np.ascontiguousarray(x[b, :, 0:HE, :], np.float32)
        bot = np.ascontiguousarray(x[b, :, H - 1:H - 1 - HE:-1, :], np.float32)
        for xec, wppc in ((top, wpp_n), (bot, wpp_f)):
            in_maps.append(dict(xe=xec, wqkT=wqkT, wvT=wvT, bqk=bqk, bv=bvf,
                                wpp=wppc, gamma=gam))
    return in_maps


def assemble(results):
    """Gather per-core [256, 64, 128] outputs into [4, 256, 128, 128]."""
    B = len(results) // 2
    y = np.empty((B, C, H, W), np.float32)
    for b in range(B):
        y[b, :, 0:64, :] = results[2 * b]["y"]
        y[b, :, 64:128, :] = results[2 * b + 1]["y"][:, ::-1, :]
    return y


def kernel(**inputs):
    r = get_runner(None)
    in_maps = make_in_maps(**inputs)
    args = r.prepare(in_maps)
    outs = r.run(args)
    return assemble(r.results(outs))
